# revision 6
# baseline (speedup 1.0000x reference)
"""Trainium2 Bass kernel for nn_GraphVertExtraLinModel.

Model (per sample n, GS=4 graph channels, M=64 nodes):
  layer: h <- max_g relu(G[n,g] @ (h @ W[g].T + b[g]))  (+ residual for l>=1)
  head:  out = relu(h @ lin1_w.T + lin1_b) @ lin2_w.T + lin2_b

Sharding: data-parallel over N=128 -> 16 samples per core, weights replicated.
No collectives needed (the max-aggregation is over GS inside each sample).

Per-core layout (tokens = 16*64 = 1024, tiled 8 x 128; h kept FEATURE-major):
  h     [p_local, (t, ptile, tok)]  one [128, 4096] tile per layer generation
  mp    [tok, p] = hT.T @ W         (mm1: lhsT = h slice, rhs = weights f32r,
                                     free dim 512 -> full PE rate)
  ms    [tok, p] = mp + b           (bias rides the PSUM->SBUF copy; bf16 out)
  xoT   [p, tok] = ms.T @ Gblk      (G-matmul flipped: ms stationary, G moving
                                     in bf16 so 128-free runs at 1 cyc/row;
                                     output lands feature-major -> NO transposes)
  h'    = relu(max_g xoT) + h       (max tree split DVE/GpSimd, fused relu via
                                     scalar_tensor_tensor)
G is pre-transposed + block-diag packed (2 samples per 128x128 tile) on host.
PE program order interleaves mm1[t] with G-matmuls[t-1] so ms copies on the
helper engines never stall the PE.
"""

import numpy as np
import ml_dtypes
from contextlib import ExitStack

import concourse.bass as bass
import concourse.tile as tile
from concourse import bacc, mybir
from concourse.bass_utils import run_bass_kernel_spmd
from concourse.alu_op_type import AluOpType

F32 = mybir.dt.float32
F32R = mybir.dt.float32r
BF16 = mybir.dt.bfloat16
RELU = mybir.ActivationFunctionType.Relu

N_CORES = 8
N_FULL = 128
N_LOC = N_FULL // N_CORES   # 16 samples per core
GS = 4
M = 64
C_IN = 128
D = 512
L = 8
TOK = N_LOC * M             # 1024 tokens per core
NT = TOK // 128             # 8 token tiles
KD = D // 128               # 4 contraction tiles for D


def _build_program():
    nc = bacc.Bacc(
        "TRN2",
        target_bir_lowering=False,
        debug=False,
        enable_asserts=False,
        num_devices=N_CORES,
    )

    xT_d = nc.dram_tensor("xT", [C_IN, TOK], F32R, kind="ExternalInput").ap()
    g_d = nc.dram_tensor("gsb", [128, GS * NT * 128], BF16, kind="ExternalInput").ap()
    w0_d = nc.dram_tensor("w0", [128, GS * D], F32R, kind="ExternalInput").ap()
    b0_d = nc.dram_tensor("b0", [128, GS * D], F32, kind="ExternalInput").ap()
    w_d = nc.dram_tensor("w", [L - 1, 128, GS * KD * D], F32R, kind="ExternalInput").ap()
    b_d = nc.dram_tensor("b", [L - 1, 128, GS * D], F32, kind="ExternalInput").ap()
    l1w_d = nc.dram_tensor("lin1", [128, KD * 128], F32R, kind="ExternalInput").ap()
    l1b_d = nc.dram_tensor("lin1b", [128, 1], F32, kind="ExternalInput").ap()
    l2w_d = nc.dram_tensor("lin2", [128, 1], F32R, kind="ExternalInput").ap()
    out_d = nc.dram_tensor("out", [1, TOK], F32, kind="ExternalOutput").ap()

    with tile.TileContext(nc) as tc, ExitStack() as ctx:
        const = ctx.enter_context(tc.tile_pool(name="const", bufs=1))
        wpool = ctx.enter_context(tc.tile_pool(name="w", bufs=2))
        bpool = ctx.enter_context(tc.tile_pool(name="b", bufs=2))
        hpool = ctx.enter_context(tc.tile_pool(name="h", bufs=2))
        mspool = ctx.enter_context(tc.tile_pool(name="ms", bufs=10))
        mtmp = ctx.enter_context(tc.tile_pool(name="mt", bufs=10))
        mpsum = ctx.enter_context(tc.tile_pool(name="mpsum", bufs=4, space="PSUM"))
        xpsum = ctx.enter_context(tc.tile_pool(name="xpsum", bufs=4, space="PSUM"))

        gsb = const.tile([128, GS * NT * 128], BF16, tag="gsb")
        nc.sync.dma_start(out=gsb[:], in_=g_d)
        xsb = const.tile([128, TOK], F32R, tag="xsb")
        nc.sync.dma_start(out=xsb[:], in_=xT_d)
        l1sb = const.tile([128, KD * 128], F32R, tag="l1w")
        nc.sync.dma_start(out=l1sb[:], in_=l1w_d)
        l1b = const.tile([128, 1], F32, tag="l1b")
        nc.sync.dma_start(out=l1b[:], in_=l1b_d)
        l2sb = const.tile([128, 1], F32R, tag="l2w")
        nc.sync.dma_start(out=l2sb[:], in_=l2w_d)

        # pending G-matmul + max-tree work: emitted one t-iteration late so the
        # PE never waits on the helper engines' ms copies
        pending = []

        def _stt_max(out_ap, a, b):
            # (a max 0) max b == max(a, b) for post-relu operands; as a
            # TensorScalarPtr it runs at 4x on DVE with all-bf16 operands
            nc.vector.scalar_tensor_tensor(
                out_ap, a, 0.0, b, op0=AluOpType.max, op1=AluOpType.max
            )

        def flush():
            # GpSimd cannot touch PSUM (and only implements Add), so the Act
            # engine relu-copies each xo bank to SBUF as bf16 (relu commutes
            # with max), the max tree runs on DVE in 4x bf16 mode, and the
            # residual add runs on GpSimd.
            if not pending:
                return
            layer, t, ms_tiles, h_new, h_prev = pending.pop()
            xrs = []
            for g in range(GS):
                xo = xpsum.tile([128, D], F32, tag="xo")
                for p in range(KD):
                    nc.tensor.matmul(
                        xo[:, p * 128 : (p + 1) * 128],
                        ms_tiles[g][:, p * 128 : (p + 1) * 128],
                        gsb[:, (g * NT + t) * 128 : (g * NT + t + 1) * 128],
                        start=True,
                        stop=True,
                    )
                xr = mtmp.tile([128, D], BF16, tag="mt")
                nc.scalar.activation(xr[:], xo[:], func=RELU)
                xrs.append(xr)
            m01 = mtmp.tile([128, D], BF16, tag="mt")
            _stt_max(m01[:], xrs[0][:], xrs[1][:])
            m23 = mtmp.tile([128, D], BF16, tag="mt")
            _stt_max(m23[:], xrs[2][:], xrs[3][:])
            hs = h_new[:, t * D : (t + 1) * D]
            if h_prev is None:
                _stt_max(hs, m01[:], m23[:])
            else:
                u = mtmp.tile([128, D], BF16, tag="mt")
                _stt_max(u[:], m01[:], m23[:])
                nc.gpsimd.tensor_tensor(
                    hs, u[:], h_prev[:, t * D : (t + 1) * D], op=AluOpType.add
                )

        h_prev = None
        for layer in range(L):
            K = 1 if layer == 0 else KD
            wsb = wpool.tile([128, GS * K * D], F32R, tag="w")
            bsb = bpool.tile([128, GS * D], F32, tag="b")
            if layer == 0:
                nc.sync.dma_start(out=wsb[:], in_=w0_d)
                nc.sync.dma_start(out=bsb[:], in_=b0_d)
            else:
                nc.sync.dma_start(out=wsb[:], in_=w_d[layer - 1])
                nc.sync.dma_start(out=bsb[:], in_=b_d[layer - 1])

            h_new = hpool.tile([128, NT * D], F32R, tag="h")
            for t in range(NT):
                ms_tiles = []
                for g in range(GS):
                    mp = mpsum.tile([128, D], F32, tag="mp")
                    if layer == 0:
                        nc.tensor.matmul(
                            mp[:],
                            xsb[:, t * 128 : (t + 1) * 128],
                            wsb[:, g * D : (g + 1) * D],
                            start=True,
                            stop=True,
                        )
                    else:
                        for c in range(KD):
                            nc.tensor.matmul(
                                mp[:],
                                h_prev[:, t * D + c * 128 : t * D + (c + 1) * 128],
                                wsb[:, (g * KD + c) * D : (g * KD + c + 1) * D],
                                start=(c == 0),
                                stop=(c == KD - 1),
                            )
                    ms = mspool.tile([128, D], BF16, tag="ms")
                    nc.vector.tensor_tensor(
                        ms[:], mp[:], bsb[:, g * D : (g + 1) * D], op=AluOpType.add
                    )
                    ms_tiles.append(ms)
                flush()
                pending.append((layer, t, ms_tiles, h_new, h_prev))
            h_prev = h_new
        flush()

        # head: p1[e, tok] = lin1 @ h.T ; x1 = relu(p1 + b1); out = lin2 @ x1
        osb = const.tile([1, TOK], F32, tag="osb")
        h3 = h_prev[:].rearrange("p (t k) -> p t k", t=NT)
        for tb in range(TOK // 512):
            p1 = mpsum.tile([128, 512], F32, tag="mp")
            for c in range(KD):
                nc.tensor.matmul(
                    p1[:],
                    l1sb[:, c * 128 : (c + 1) * 128],
                    h3[:, tb * 4 : (tb + 1) * 4, c * 128 : (c + 1) * 128],
                    start=(c == 0),
                    stop=(c == KD - 1),
                )
            x1 = mtmp.tile([128, 512], F32R, tag="mt")
            nc.scalar.activation(x1[:], p1[:], func=RELU, bias=l1b[:])
            p2 = xpsum.tile([1, 512], F32, tag="xo")
            nc.tensor.matmul(p2[:], l2sb[:], x1[:], start=True, stop=True)
            nc.vector.tensor_copy(osb[0:1, tb * 512 : (tb + 1) * 512], p2[:])
        nc.sync.dma_start(out=out_d[:], in_=osb[:])

    nc.compile()
    return nc


_NC = None


def _get_nc():
    global _NC
    if _NC is None:
        _NC = _build_program()
    return _NC


def _prep_in_maps(G, x, W0, b0, W, b, lin1_w, lin1_b, lin2_w, lin2_b):
    G = np.ascontiguousarray(np.asarray(G, dtype=np.float32))
    x = np.ascontiguousarray(np.asarray(x, dtype=np.float32))
    W0 = np.asarray(W0, dtype=np.float32)
    b0 = np.asarray(b0, dtype=np.float32)
    W = np.asarray(W, dtype=np.float32)
    b = np.asarray(b, dtype=np.float32)
    lin1_w = np.asarray(lin1_w, dtype=np.float32)
    lin1_b = np.asarray(lin1_b, dtype=np.float32)
    lin2_w = np.asarray(lin2_w, dtype=np.float32)

    # shared (replicated) tensors, host pre-arranged into SBUF layout
    # w0: [c_local, (g, p)] with row = input channel c
    w0f = np.ascontiguousarray(W0.transpose(2, 0, 1).reshape(C_IN, GS * D))
    b0f = np.ascontiguousarray(
        np.broadcast_to(b0.reshape(1, GS * D), (128, GS * D))
    ).astype(np.float32)
    # w: [l, c_local, (g, ctile, p)]; W[l,g,p,c] -> rows c_local of ctile
    wf = np.ascontiguousarray(
        W.reshape(L - 1, GS, D, KD, 128).transpose(0, 4, 1, 3, 2).reshape(
            L - 1, 128, GS * KD * D
        )
    )
    bf = np.ascontiguousarray(
        np.broadcast_to(b.reshape(L - 1, 1, GS * D), (L - 1, 128, GS * D))
    ).astype(np.float32)
    # lin1: [c_local, (ctile, e)]
    l1f = np.ascontiguousarray(
        lin1_w.T.reshape(KD, 128, 128).transpose(1, 0, 2).reshape(128, KD * 128)
    )
    l1b = np.ascontiguousarray(lin1_b.reshape(128, 1))
    l2f = np.ascontiguousarray(lin2_w.T)  # [128, 1]

    in_maps = []
    for cix in range(N_CORES):
        Gs = G[cix * N_LOC : (cix + 1) * N_LOC]                      # [16,4,64,64]
        xs = x[cix * N_LOC : (cix + 1) * N_LOC]                      # [16,64,128]
        xT = np.ascontiguousarray(xs.reshape(TOK, C_IN).T)           # [128,1024]
        Gt = Gs.transpose(1, 0, 3, 2)                                # [4,16,64j,64i]
        gblk = np.zeros((GS, NT, 128, 128), np.float32)
        gblk[:, :, 0:64, 0:64] = Gt[:, 0::2]
        gblk[:, :, 64:128, 64:128] = Gt[:, 1::2]
        gf = np.ascontiguousarray(
            gblk.transpose(2, 0, 1, 3).reshape(128, GS * NT * 128)
        ).astype(ml_dtypes.bfloat16)
        in_maps.append(
            {
                "xT": xT,
                "gsb": gf,
                "w0": w0f,
                "b0": b0f,
                "w": wf,
                "b": bf,
                "lin1": l1f,
                "lin1b": l1b,
                "lin2": l2f,
            }
        )

    return in_maps


def kernel(G, x, W0, b0, W, b, lin1_w, lin1_b, lin2_w, lin2_b, _trace=False):
    lin2_b = np.asarray(lin2_b, dtype=np.float32)
    in_maps = _prep_in_maps(G, x, W0, b0, W, b, lin1_w, lin1_b, lin2_w, lin2_b)
    res = run_bass_kernel_spmd(_get_nc(), in_maps, list(range(N_CORES)), trace=_trace)
    kernel._last_results = res
    out = np.concatenate(
        [res.results[c]["out"].reshape(N_LOC, M, 1) for c in range(N_CORES)], axis=0
    )
    return (out + lin2_b[0]).astype(np.float32)


# revision 7
# speedup vs baseline: 1.0673x; 1.0673x over previous
"""Trainium2 Bass kernel for nn_GraphVertExtraLinModel.

Model (per sample n, GS=4 graph channels, M=64 nodes):
  layer: h <- max_g relu(G[n,g] @ (h @ W[g].T + b[g]))  (+ residual for l>=1)
  head:  out = relu(h @ lin1_w.T + lin1_b) @ lin2_w.T + lin2_b

Sharding: data-parallel over N=128 -> 16 samples per core, weights replicated.
No collectives needed (the max-aggregation is over GS inside each sample).

Per-core layout (tokens = 16*64 = 1024, tiled 8 x 128; h kept FEATURE-major):
  mp    [tok, (g-pair, p)] = hT.T @ W   (mm1: lhsT = h slice, rhs = weights
                                         f32r, free dim 512 -> full PE rate;
                                         two g channels packed per 2-bank PSUM
                                         tile to halve elementwise op count)
  ms    [tok, p] = mp + b               (bias rides the PSUM->SBUF copy, bf16;
                                         pair 0|1 on DVE, pair 2|3 via
                                         Act-copy + GpSimd-add chain)
  xoT   [p, tok] = ms.T @ Gblk          (G-matmul flipped: ms stationary, G
                                         moving in bf16 so the 128-free matmul
                                         runs 1 cyc/row; output lands feature-
                                         major -> NO transposes anywhere)
  xr    = relu(xoT) bf16 on Act         (relu commutes with max)
  h'    = max_g xr + h                  (bf16 max tree on DVE 2x mode,
                                         residual add on GpSimd)
G is pre-transposed + block-diag packed (2 samples per 128x128 tile) on host.
PE program order interleaves mm1[t] with the G-matmuls of t-1 so the helper
engines' ms copies never stall the PE.
"""

import numpy as np
import ml_dtypes
from contextlib import ExitStack

import concourse.bass as bass
import concourse.tile as tile
from concourse import bacc, mybir
from concourse.bass_utils import run_bass_kernel_spmd
from concourse.alu_op_type import AluOpType

F32 = mybir.dt.float32
F32R = mybir.dt.float32r
BF16 = mybir.dt.bfloat16
RELU = mybir.ActivationFunctionType.Relu
COPY = mybir.ActivationFunctionType.Copy

N_CORES = 8
N_FULL = 128
N_LOC = N_FULL // N_CORES   # 16 samples per core
GS = 4
M = 64
C_IN = 128
D = 512
L = 8
TOK = N_LOC * M             # 1024 tokens per core
NT = TOK // 128             # 8 token tiles
KD = D // 128               # 4 contraction tiles for D
D2 = 2 * D                  # paired g-channel width


def _build_program():
    nc = bacc.Bacc(
        "TRN2",
        target_bir_lowering=False,
        debug=False,
        enable_asserts=False,
        num_devices=N_CORES,
    )

    xT_d = nc.dram_tensor("xT", [C_IN, TOK], F32R, kind="ExternalInput").ap()
    g_d = nc.dram_tensor("gsb", [128, GS * NT * 128], BF16, kind="ExternalInput").ap()
    w0_d = nc.dram_tensor("w0", [128, GS * D], F32R, kind="ExternalInput").ap()
    b0_d = nc.dram_tensor("b0", [128, GS * D], BF16, kind="ExternalInput").ap()
    w_d = nc.dram_tensor("w", [L - 1, 128, GS * KD * D], F32R, kind="ExternalInput").ap()
    b_d = nc.dram_tensor("b", [L - 1, 128, GS * D], BF16, kind="ExternalInput").ap()
    l1w_d = nc.dram_tensor("lin1", [128, KD * 128], F32R, kind="ExternalInput").ap()
    l1b_d = nc.dram_tensor("lin1b", [128, 1], F32, kind="ExternalInput").ap()
    l2w_d = nc.dram_tensor("lin2", [128, 1], F32R, kind="ExternalInput").ap()
    out_d = nc.dram_tensor("out", [1, TOK], F32, kind="ExternalOutput").ap()

    with tile.TileContext(nc) as tc, ExitStack() as ctx:
        const = ctx.enter_context(tc.tile_pool(name="const", bufs=1))
        wpool = ctx.enter_context(tc.tile_pool(name="w", bufs=2))
        bpool = ctx.enter_context(tc.tile_pool(name="b", bufs=2))
        hpool = ctx.enter_context(tc.tile_pool(name="h", bufs=2))
        mspool = ctx.enter_context(tc.tile_pool(name="ms", bufs=5))
        mtmp = ctx.enter_context(tc.tile_pool(name="mt", bufs=8))
        mpsum = ctx.enter_context(tc.tile_pool(name="mpsum", bufs=2, space="PSUM"))
        xpsum = ctx.enter_context(tc.tile_pool(name="xpsum", bufs=2, space="PSUM"))

        # startup-critical DMAs first: the first mm1 needs xT and w0[g0|g1]
        xsb = const.tile([128, TOK], F32R, tag="xsb")
        nc.sync.dma_start(out=xsb[:], in_=xT_d)
        wsb0 = wpool.tile([128, GS * D], F32R, tag="w")
        nc.sync.dma_start(out=wsb0[:, 0:D2], in_=w0_d[:, 0:D2])
        nc.sync.dma_start(out=wsb0[:, D2 : 2 * D2], in_=w0_d[:, D2 : 2 * D2])
        bsb0 = bpool.tile([128, GS * D], BF16, tag="b")
        nc.sync.dma_start(out=bsb0[:], in_=b0_d)
        gsb = const.tile([128, GS * NT * 128], BF16, tag="gsb")
        nc.sync.dma_start(out=gsb[:], in_=g_d)
        l1sb = const.tile([128, KD * 128], F32R, tag="l1w")
        nc.sync.dma_start(out=l1sb[:], in_=l1w_d)
        l1b = const.tile([128, 1], F32, tag="l1b")
        nc.sync.dma_start(out=l1b[:], in_=l1b_d)
        l2sb = const.tile([128, 1], F32R, tag="l2w")
        nc.sync.dma_start(out=l2sb[:], in_=l2w_d)
        osb = const.tile([1, TOK], F32, tag="osb")

        # pending G-matmul + max-tree work: emitted one t-iteration late so the
        # PE never waits on the helper engines' ms copies
        pending = []

        def flush():
            if not pending:
                return
            layer, t, ms_pair, h_new, h_prev = pending.pop()
            xrs = []
            for half in range(2):          # halves: (g0|g1), (g2|g3)
                xo = xpsum.tile([128, D2], F32, tag="xo")
                for gi in range(2):
                    g = half * 2 + gi
                    for p in range(KD):
                        nc.tensor.matmul(
                            xo[:, gi * D + p * 128 : gi * D + (p + 1) * 128],
                            ms_pair[half][:, gi * D + p * 128 : gi * D + (p + 1) * 128],
                            gsb[:, (g * NT + t) * 128 : (g * NT + t + 1) * 128],
                            start=True,
                            stop=True,
                        )
                xr = mtmp.tile([128, D2], BF16, tag="mt")
                nc.scalar.activation(xr[:], xo[:], func=RELU)
                xrs.append(xr)
            m01 = mtmp.tile([128, D], BF16, tag="mt")
            nc.vector.tensor_tensor(
                m01[:], xrs[0][:, 0:D], xrs[0][:, D:D2], op=AluOpType.max
            )
            m23 = mtmp.tile([128, D], BF16, tag="mt")
            nc.vector.tensor_tensor(
                m23[:], xrs[1][:, 0:D], xrs[1][:, D:D2], op=AluOpType.max
            )
            hs = h_new[:, t * D : (t + 1) * D]
            if h_prev is None:
                nc.vector.tensor_tensor(hs, m01[:], m23[:], op=AluOpType.max)
            else:
                u = mtmp.tile([128, D], BF16, tag="mt")
                nc.vector.tensor_tensor(u[:], m01[:], m23[:], op=AluOpType.max)
                nc.gpsimd.tensor_tensor(
                    hs, u[:], h_prev[:, t * D : (t + 1) * D], op=AluOpType.add
                )

        def emit_head_block(h_tile, tb):
            h3 = h_tile[:].rearrange("p (t k) -> p t k", t=NT)
            p1 = mpsum.tile([128, 512], F32, tag="mp")
            for c in range(KD):
                nc.tensor.matmul(
                    p1[:],
                    l1sb[:, c * 128 : (c + 1) * 128],
                    h3[:, tb * 4 : (tb + 1) * 4, c * 128 : (c + 1) * 128],
                    start=(c == 0),
                    stop=(c == KD - 1),
                )
            x1 = mtmp.tile([128, 512], F32R, tag="mt")
            nc.scalar.activation(x1[:], p1[:], func=RELU, bias=l1b[:])
            p2 = xpsum.tile([1, 512], F32, tag="xo")
            nc.tensor.matmul(p2[:], l2sb[:], x1[:], start=True, stop=True)
            nc.vector.tensor_copy(osb[0:1, tb * 512 : (tb + 1) * 512], p2[:])

        h_prev = None
        for layer in range(L):
            K = 1 if layer == 0 else KD
            if layer == 0:
                wsb, bsb = wsb0, bsb0
            else:
                wsb = wpool.tile([128, GS * K * D], F32R, tag="w")
                nc.sync.dma_start(out=wsb[:], in_=w_d[layer - 1])
                bsb = bpool.tile([128, GS * D], BF16, tag="b")
                nc.sync.dma_start(out=bsb[:], in_=b_d[layer - 1])

            h_new = hpool.tile([128, NT * D], F32R, tag="h")
            for t in range(NT):
                ms_pair = []
                for half in range(2):
                    mp = mpsum.tile([128, D2], F32, tag="mp")
                    for gi in range(2):
                        g = half * 2 + gi
                        if layer == 0:
                            nc.tensor.matmul(
                                mp[:, gi * D : (gi + 1) * D],
                                xsb[:, t * 128 : (t + 1) * 128],
                                wsb[:, g * D : (g + 1) * D],
                                start=True,
                                stop=True,
                            )
                        else:
                            for c in range(KD):
                                nc.tensor.matmul(
                                    mp[:, gi * D : (gi + 1) * D],
                                    h_prev[:, t * D + c * 128 : t * D + (c + 1) * 128],
                                    wsb[:, (g * KD + c) * D : (g * KD + c + 1) * D],
                                    start=(c == 0),
                                    stop=(c == KD - 1),
                                )
                    ms = mspool.tile([128, D2], BF16, tag="ms")
                    if half == 0:
                        nc.vector.tensor_tensor(
                            ms[:], mp[:], bsb[:, 0:D2], op=AluOpType.add
                        )
                    else:
                        msf = mtmp.tile([128, D2], BF16, tag="mt")
                        nc.scalar.activation(msf[:], mp[:], func=COPY)
                        nc.gpsimd.tensor_tensor(
                            ms[:], msf[:], bsb[:, D2 : 2 * D2], op=AluOpType.add
                        )
                    ms_pair.append(ms)
                flush()
                if layer == L - 1 and t == NT - 3:
                    emit_head_block(h_new, 0)  # h slices t0..3 are complete
                pending.append((layer, t, ms_pair, h_new, h_prev))
            h_prev = h_new
        flush()
        emit_head_block(h_prev, 1)
        nc.sync.dma_start(out=out_d[:], in_=osb[:])

    nc.compile()
    return nc


_NC = None


def _get_nc():
    global _NC
    if _NC is None:
        _NC = _build_program()
    return _NC


def _prep_in_maps(G, x, W0, b0, W, b, lin1_w, lin1_b, lin2_w, lin2_b):
    BF = ml_dtypes.bfloat16
    G = np.ascontiguousarray(np.asarray(G, dtype=np.float32))
    x = np.ascontiguousarray(np.asarray(x, dtype=np.float32))
    W0 = np.asarray(W0, dtype=np.float32)
    b0 = np.asarray(b0, dtype=np.float32)
    W = np.asarray(W, dtype=np.float32)
    b = np.asarray(b, dtype=np.float32)
    lin1_w = np.asarray(lin1_w, dtype=np.float32)
    lin1_b = np.asarray(lin1_b, dtype=np.float32)
    lin2_w = np.asarray(lin2_w, dtype=np.float32)

    # shared (replicated) tensors, host pre-arranged into SBUF layout
    # w0: [c_local, (g, p)] with row = input channel c
    w0f = np.ascontiguousarray(W0.transpose(2, 0, 1).reshape(C_IN, GS * D))
    b0f = np.ascontiguousarray(
        np.broadcast_to(b0.reshape(1, GS * D), (128, GS * D))
    ).astype(BF)
    # w: [l, c_local, (g, ctile, p)]; W[l,g,p,c] -> rows c_local of ctile
    wf = np.ascontiguousarray(
        W.reshape(L - 1, GS, D, KD, 128).transpose(0, 4, 1, 3, 2).reshape(
            L - 1, 128, GS * KD * D
        )
    )
    bf = np.ascontiguousarray(
        np.broadcast_to(b.reshape(L - 1, 1, GS * D), (L - 1, 128, GS * D))
    ).astype(BF)
    # lin1: [c_local, (ctile, e)]
    l1f = np.ascontiguousarray(
        lin1_w.T.reshape(KD, 128, 128).transpose(1, 0, 2).reshape(128, KD * 128)
    )
    l1b = np.ascontiguousarray(lin1_b.reshape(128, 1))
    l2f = np.ascontiguousarray(lin2_w.T)  # [128, 1]

    in_maps = []
    for cix in range(N_CORES):
        Gs = G[cix * N_LOC : (cix + 1) * N_LOC]                      # [16,4,64,64]
        xs = x[cix * N_LOC : (cix + 1) * N_LOC]                      # [16,64,128]
        xT = np.ascontiguousarray(xs.reshape(TOK, C_IN).T)           # [128,1024]
        Gt = Gs.transpose(1, 0, 3, 2)                                # [4,16,64j,64i]
        gblk = np.zeros((GS, NT, 128, 128), np.float32)
        gblk[:, :, 0:64, 0:64] = Gt[:, 0::2]
        gblk[:, :, 64:128, 64:128] = Gt[:, 1::2]
        gf = np.ascontiguousarray(
            gblk.transpose(2, 0, 1, 3).reshape(128, GS * NT * 128)
        ).astype(BF)
        in_maps.append(
            {
                "xT": xT,
                "gsb": gf,
                "w0": w0f,
                "b0": b0f,
                "w": wf,
                "b": bf,
                "lin1": l1f,
                "lin1b": l1b,
                "lin2": l2f,
            }
        )

    return in_maps


def kernel(G, x, W0, b0, W, b, lin1_w, lin1_b, lin2_w, lin2_b, _trace=False):
    lin2_b = np.asarray(lin2_b, dtype=np.float32)
    in_maps = _prep_in_maps(G, x, W0, b0, W, b, lin1_w, lin1_b, lin2_w, lin2_b)
    res = run_bass_kernel_spmd(_get_nc(), in_maps, list(range(N_CORES)), trace=_trace)
    kernel._last_results = res
    out = np.concatenate(
        [res.results[c]["out"].reshape(N_LOC, M, 1) for c in range(N_CORES)], axis=0
    )
    return (out + lin2_b[0]).astype(np.float32)


# revision 31
# speedup vs baseline: 1.0901x; 1.0213x over previous
"""Trainium2 Bass kernel for nn_GraphVertExtraLinModel.

Model (per sample n, GS=4 graph channels, M=64 nodes):
  layer: h <- max_g relu(G[n,g] @ (h @ W[g].T + b[g]))  (+ residual for l>=1)
  head:  out = relu(h @ lin1_w.T + lin1_b) @ lin2_w.T + lin2_b

Sharding: data-parallel over N=128 -> 16 samples per core, weights replicated.
No collectives needed (the max-aggregation is over GS inside each sample).

Per-core layout (tokens = 16*64 = 1024, tiled 8 x 128; h kept FEATURE-major):
  mp    [tok, (g-pair, p)] = hT.T @ W   (mm1: lhsT = h slice, rhs = weights
                                         f32r, free dim 512 -> full PE rate;
                                         two g channels packed per 2-bank PSUM
                                         tile to halve elementwise op count)
  ms    [tok, p] = mp + b               (bias rides the PSUM->SBUF copy, bf16;
                                         pair 0|1 on DVE, pair 2|3 via
                                         Act-copy + GpSimd-add chain)
  xoT   [p, tok] = ms.T @ Gblk          (G-matmul flipped: ms stationary, G
                                         moving in bf16 so the 128-free matmul
                                         runs 1 cyc/row; output lands feature-
                                         major -> NO transposes anywhere)
  xr    = relu(xoT) bf16 on Act         (relu commutes with max)
  h'    = max_g xr + h                  (bf16 max tree on DVE 2x mode,
                                         residual add on GpSimd)
G is pre-transposed + block-diag packed (2 samples per 128x128 tile) on host.
PE program order interleaves mm1[t] with the G-matmuls of t-1 so the helper
engines' ms copies never stall the PE.
"""

import numpy as np
import ml_dtypes
from contextlib import ExitStack

import concourse.bass as bass
import concourse.tile as tile
from concourse import bacc, mybir
from concourse.bass_utils import run_bass_kernel_spmd
from concourse.alu_op_type import AluOpType

F32 = mybir.dt.float32
F32R = mybir.dt.float32r
BF16 = mybir.dt.bfloat16
RELU = mybir.ActivationFunctionType.Relu
COPY = mybir.ActivationFunctionType.Copy

N_CORES = 8
N_FULL = 128
N_LOC = N_FULL // N_CORES   # 16 samples per core
GS = 4
M = 64
C_IN = 128
D = 512
L = 8
TOK = N_LOC * M             # 1024 tokens per core
NT = TOK // 128             # 8 token tiles
KD = D // 128               # 4 contraction tiles for D
D2 = 2 * D                  # paired g-channel width


def _build_program():
    nc = bacc.Bacc(
        "TRN2",
        target_bir_lowering=False,
        debug=False,
        enable_asserts=False,
        num_devices=N_CORES,
    )

    xT_d = nc.dram_tensor("xT", [C_IN, TOK], F32R, kind="ExternalInput").ap()
    g_d = nc.dram_tensor("gsb", [128, GS * NT * 128], BF16, kind="ExternalInput").ap()
    w0_d = nc.dram_tensor("w0", [128, GS * D], F32R, kind="ExternalInput").ap()
    b0_d = nc.dram_tensor("b0", [128, GS * D], BF16, kind="ExternalInput").ap()
    w_d = nc.dram_tensor("w", [L - 1, 128, GS * KD * D], F32R, kind="ExternalInput").ap()
    b_d = nc.dram_tensor("b", [L - 1, 128, GS * D], BF16, kind="ExternalInput").ap()
    l1w_d = nc.dram_tensor("lin1", [128, KD * 128], F32R, kind="ExternalInput").ap()
    l1b_d = nc.dram_tensor("lin1b", [128, 1], F32, kind="ExternalInput").ap()
    l2w_d = nc.dram_tensor("lin2", [128, 1], F32R, kind="ExternalInput").ap()
    out_d = nc.dram_tensor("out", [1, TOK], F32, kind="ExternalOutput").ap()

    with tile.TileContext(nc) as tc, ExitStack() as ctx:
        const = ctx.enter_context(tc.tile_pool(name="const", bufs=1))
        wpool = ctx.enter_context(tc.tile_pool(name="w", bufs=2))
        bpool = ctx.enter_context(tc.tile_pool(name="b", bufs=2))
        hpool = ctx.enter_context(tc.tile_pool(name="h", bufs=2))
        mspool = ctx.enter_context(tc.tile_pool(name="ms", bufs=6))
        mtmp = ctx.enter_context(tc.tile_pool(name="mt", bufs=10))
        mpsum = ctx.enter_context(tc.tile_pool(name="mpsum", bufs=2, space="PSUM"))
        xpsum = ctx.enter_context(tc.tile_pool(name="xpsum", bufs=2, space="PSUM"))

        # startup-critical DMAs first: the first mm1 needs xT and w0[g0|g1];
        # gsb is needed by the first G-matmul flush (~2 t-iterations in)
        xsb = const.tile([128, TOK], F32R, tag="xsb")
        nc.sync.dma_start(out=xsb[:, 0:256], in_=xT_d[:, 0:256])
        wsb0 = wpool.tile([128, GS * D], F32R, tag="w")
        bsb0 = bpool.tile([128, GS * D], BF16, tag="b")
        gsb = const.tile([128, GS * NT * 128], BF16, tag="gsb")
        for g in range(GS):
            nc.sync.dma_start(
                out=wsb0[:, g * D : (g + 1) * D], in_=w0_d[:, g * D : (g + 1) * D]
            )
            if g == 1:
                nc.sync.dma_start(out=xsb[:, 256:TOK], in_=xT_d[:, 256:TOK])
        nc.sync.dma_start(out=bsb0[:], in_=b0_d)
        # gsb is t-major [(t, g, i)]: the first flush only needs the t=0 chunk
        nc.sync.dma_start(out=gsb[:, 0:1024], in_=g_d[:, 0:1024])
        nc.sync.dma_start(out=gsb[:, 1024:], in_=g_d[:, 1024:])
        l1sb = const.tile([128, KD * 128], F32R, tag="l1w")
        l1b = const.tile([128, 1], F32, tag="l1b")
        l2sb = const.tile([128, 1], F32R, tag="l2w")
        osb = const.tile([1, TOK], F32, tag="osb")

        # pending G-matmul + max-tree work: emitted one t-iteration late so the
        # PE never waits on the helper engines' ms copies
        pending = []

        def flush():
            if not pending:
                return
            layer, t, ms_pair, h_new, h_prev = pending.pop()
            xrs = []
            for half in range(2):          # halves: (g0|g1), (g2|g3)
                xo = xpsum.tile([128, D2], F32, tag="xo")
                for gi in range(2):
                    g = half * 2 + gi
                    for p in range(KD):
                        nc.tensor.matmul(
                            xo[:, gi * D + p * 128 : gi * D + (p + 1) * 128],
                            ms_pair[half][:, gi * D + p * 128 : gi * D + (p + 1) * 128],
                            gsb[:, (t * GS + g) * 128 : (t * GS + g + 1) * 128],
                            start=True,
                            stop=True,
                        )
                xr = mtmp.tile([128, D2], BF16, tag="mt")
                nc.scalar.activation(xr[:], xo[:], func=RELU)
                xrs.append(xr)
            m01 = mtmp.tile([128, D], BF16, tag="mt")
            nc.vector.tensor_tensor(
                m01[:], xrs[0][:, 0:D], xrs[0][:, D:D2], op=AluOpType.max
            )
            m23 = mtmp.tile([128, D], BF16, tag="mt")
            nc.vector.tensor_tensor(
                m23[:], xrs[1][:, 0:D], xrs[1][:, D:D2], op=AluOpType.max
            )
            hs = h_new[:, t * D : (t + 1) * D]
            if h_prev is None:
                nc.vector.tensor_tensor(hs, m01[:], m23[:], op=AluOpType.max)
            else:
                u = mtmp.tile([128, D], BF16, tag="mt")
                nc.vector.tensor_tensor(u[:], m01[:], m23[:], op=AluOpType.max)
                nc.vector.tensor_tensor(
                    hs, u[:], h_prev[:, t * D : (t + 1) * D], op=AluOpType.add
                )

        def emit_head_block(h_tile, tb):
            # 256-token head block (2 t-tiles): free dim stays >= 256 so the
            # f32r matmuls run at full rate; small blocks interleave into the
            # layer-7 flush stream so the tail drain is just one block deep
            h3 = h_tile[:].rearrange("p (t k) -> p t k", t=NT)
            p1 = mpsum.tile([128, 256], F32, tag="mp")
            for c in range(KD):
                nc.tensor.matmul(
                    p1[:],
                    l1sb[:, c * 128 : (c + 1) * 128],
                    h3[:, tb * 2 : (tb + 1) * 2, c * 128 : (c + 1) * 128],
                    start=(c == 0),
                    stop=(c == KD - 1),
                )
            x1 = mtmp.tile([128, 256], F32R, tag="mt")
            nc.scalar.activation(x1[:], p1[:], func=RELU, bias=l1b[:])
            p2 = xpsum.tile([1, 256], F32, tag="xo")
            nc.tensor.matmul(p2[:], l2sb[:], x1[:], start=True, stop=True)
            nc.vector.tensor_copy(osb[0:1, tb * 256 : (tb + 1) * 256], p2[:])

        h_prev = None
        for layer in range(L):
            K = 1 if layer == 0 else KD
            if layer == 0:
                wsb, bsb = wsb0, bsb0
            else:
                # per-g chunks so the first mm1 of the layer isn't gated on
                # the full 11.7us weight transfer
                wsb = wpool.tile([128, GS * K * D], F32R, tag="w")
                for g in range(GS):
                    nc.sync.dma_start(
                        out=wsb[:, g * K * D : (g + 1) * K * D],
                        in_=w_d[layer - 1][:, g * K * D : (g + 1) * K * D],
                    )
                bsb = bpool.tile([128, GS * D], BF16, tag="b")
                nc.sync.dma_start(out=bsb[:], in_=b_d[layer - 1])
            if layer == 1:
                # head weights: needed only at the very end; keep them out of
                # the startup DMA queue
                nc.sync.dma_start(out=l1sb[:], in_=l1w_d)
                nc.sync.dma_start(out=l1b[:], in_=l1b_d)
                nc.sync.dma_start(out=l2sb[:], in_=l2w_d)

            h_new = hpool.tile([128, NT * D], F32R, tag="h")
            for t in range(NT):
                ms_pair = []
                for half in range(2):
                    mp = mpsum.tile([128, D2], F32, tag="mp")
                    for gi in range(2):
                        g = half * 2 + gi
                        if layer == 0:
                            nc.tensor.matmul(
                                mp[:, gi * D : (gi + 1) * D],
                                xsb[:, t * 128 : (t + 1) * 128],
                                wsb[:, g * D : (g + 1) * D],
                                start=True,
                                stop=True,
                            )
                        else:
                            for c in range(KD):
                                nc.tensor.matmul(
                                    mp[:, gi * D : (gi + 1) * D],
                                    h_prev[:, t * D + c * 128 : t * D + (c + 1) * 128],
                                    wsb[:, (g * KD + c) * D : (g * KD + c + 1) * D],
                                    start=(c == 0),
                                    stop=(c == KD - 1),
                                )
                    ms = mspool.tile([128, D2], BF16, tag="ms")
                    last_tile = layer == L - 1 and t == NT - 1
                    if half == 0 or last_tile:
                        # last tile: DVE's shorter latency chain matters more
                        # than steady-state engine balance
                        nc.vector.tensor_tensor(
                            ms[:],
                            mp[:],
                            bsb[:, half * D2 : (half + 1) * D2],
                            op=AluOpType.add,
                        )
                    else:
                        msf = mtmp.tile([128, D2], BF16, tag="mt")
                        nc.scalar.activation(msf[:], mp[:], func=COPY)
                        nc.gpsimd.tensor_tensor(
                            ms[:], msf[:], bsb[:, D2 : 2 * D2], op=AluOpType.add
                        )
                    ms_pair.append(ms)
                flush()
                pending.append((layer, t, ms_pair, h_new, h_prev))
            h_prev = h_new
        flush()
        for tb in range(4):
            emit_head_block(h_prev, tb)
        nc.sync.dma_start(out=out_d[:], in_=osb[:])

    nc.compile()
    return nc


_NC = None


def _get_nc():
    global _NC
    if _NC is None:
        _NC = _build_program()
    return _NC


def _prep_in_maps(G, x, W0, b0, W, b, lin1_w, lin1_b, lin2_w, lin2_b):
    BF = ml_dtypes.bfloat16
    G = np.ascontiguousarray(np.asarray(G, dtype=np.float32))
    x = np.ascontiguousarray(np.asarray(x, dtype=np.float32))
    W0 = np.asarray(W0, dtype=np.float32)
    b0 = np.asarray(b0, dtype=np.float32)
    W = np.asarray(W, dtype=np.float32)
    b = np.asarray(b, dtype=np.float32)
    lin1_w = np.asarray(lin1_w, dtype=np.float32)
    lin1_b = np.asarray(lin1_b, dtype=np.float32)
    lin2_w = np.asarray(lin2_w, dtype=np.float32)

    # shared (replicated) tensors, host pre-arranged into SBUF layout
    # w0: [c_local, (g, p)] with row = input channel c
    w0f = np.ascontiguousarray(W0.transpose(2, 0, 1).reshape(C_IN, GS * D))
    b0f = np.ascontiguousarray(
        np.broadcast_to(b0.reshape(1, GS * D), (128, GS * D))
    ).astype(BF)
    # w: [l, c_local, (g, ctile, p)]; W[l,g,p,c] -> rows c_local of ctile
    wf = np.ascontiguousarray(
        W.reshape(L - 1, GS, D, KD, 128).transpose(0, 4, 1, 3, 2).reshape(
            L - 1, 128, GS * KD * D
        )
    )
    bf = np.ascontiguousarray(
        np.broadcast_to(b.reshape(L - 1, 1, GS * D), (L - 1, 128, GS * D))
    ).astype(BF)
    # lin1: [c_local, (ctile, e)]
    l1f = np.ascontiguousarray(
        lin1_w.T.reshape(KD, 128, 128).transpose(1, 0, 2).reshape(128, KD * 128)
    )
    l1b = np.ascontiguousarray(lin1_b.reshape(128, 1))
    l2f = np.ascontiguousarray(lin2_w.T)  # [128, 1]

    in_maps = []
    for cix in range(N_CORES):
        Gs = G[cix * N_LOC : (cix + 1) * N_LOC]                      # [16,4,64,64]
        xs = x[cix * N_LOC : (cix + 1) * N_LOC]                      # [16,64,128]
        xT = np.ascontiguousarray(xs.reshape(TOK, C_IN).T)           # [128,1024]
        Gt = Gs.transpose(1, 0, 3, 2)                                # [4,16,64j,64i]
        gblk = np.zeros((GS, NT, 128, 128), np.float32)
        gblk[:, :, 0:64, 0:64] = Gt[:, 0::2]
        gblk[:, :, 64:128, 64:128] = Gt[:, 1::2]
        gf = np.ascontiguousarray(
            gblk.transpose(2, 1, 0, 3).reshape(128, NT * GS * 128)
        ).astype(BF)
        in_maps.append(
            {
                "xT": xT,
                "gsb": gf,
                "w0": w0f,
                "b0": b0f,
                "w": wf,
                "b": bf,
                "lin1": l1f,
                "lin1b": l1b,
                "lin2": l2f,
            }
        )

    return in_maps


def kernel(G, x, W0, b0, W, b, lin1_w, lin1_b, lin2_w, lin2_b, _trace=False):
    lin2_b = np.asarray(lin2_b, dtype=np.float32)
    in_maps = _prep_in_maps(G, x, W0, b0, W, b, lin1_w, lin1_b, lin2_w, lin2_b)
    res = run_bass_kernel_spmd(_get_nc(), in_maps, list(range(N_CORES)), trace=_trace)
    kernel._last_results = res
    out = np.concatenate(
        [res.results[c]["out"].reshape(N_LOC, M, 1) for c in range(N_CORES)], axis=0
    )
    return (out + lin2_b[0]).astype(np.float32)
